# revision 2
# baseline (speedup 1.0000x reference)
"""Trainium2 Bass kernel for nn_CouplingNSF (coupling-layer neural spline flow).

Contract: kernel(**inputs) takes the FULL inputs (x [131072,64], W1..W4, b1..b4)
and returns (concat([x1, y2], axis=1) [131072,64] float32, log_det [131072] float32),
matching reference.reference().

Strategy: pure data-parallel over 8 NeuronCores (16384 rows each).
 - MLP runs feature-major (activations [features, rows]) so weights are the
   stationary matmul operand and no transposes are needed between layers;
   x1^T is prepared on the host.  The last layer uses lhsT = h3^T which flips
   the output to row-major [rows, 736]; b4 is added with a contraction-1
   ones-matmul accumulated into the same PSUM tile.
 - Rational-quadratic spline per (row, dim) in row-major layout:
   cumsum over the 8 bins via tensor_tensor_scan with a 0/1 reset mask,
   bin search via one broadcast compare, and all 7-value gathers via
   "select-scan": state = max(lt*state, Q - 1000*lt) walks each 8-group and
   ends at Q[bin].
"""

import numpy as np

DIM = 64
DH = 32          # d_half
K = 8
B_TAIL = 3.0
HID = 128
BATCH = 131072
MIN_W = 1e-3
MIN_H = 1e-3
MIN_D = 1e-3
N_CORES = 8
R = BATCH // N_CORES          # rows per core = 16384
T = 4                         # row-tiles (of 128) per chunk
CHUNK = 128 * T               # 512 rows
N_CHUNKS = R // CHUNK         # 32

C_W = 2 * B_TAIL * (1.0 - MIN_W * K)   # 6*(1-8e-3)
C_H = 2 * B_TAIL * (1.0 - MIN_H * K)
D_BOUND = 1.0 - MIN_D                  # stored boundary value (MIN added post-gather)
BIG = 1000.0

_CACHE = {}


def _build(n_chunks=N_CHUNKS):
    import concourse.bass as bass
    from concourse import bacc, mybir
    from concourse.tile import TileContext

    f32 = mybir.dt.float32
    u8 = mybir.dt.uint8
    Alu = mybir.AluOpType
    Act = mybir.ActivationFunctionType

    rows = n_chunks * CHUNK

    nc = bacc.Bacc("TRN2", target_bir_lowering=False, debug=False,
                   num_devices=N_CORES)

    x1t_d = nc.dram_tensor("x1t", [DH, rows], f32, kind="ExternalInput").ap()
    x2_d = nc.dram_tensor("x2", [rows, DH], f32, kind="ExternalInput").ap()
    w1_d = nc.dram_tensor("w1", [DH, HID], f32, kind="ExternalInput").ap()
    w2_d = nc.dram_tensor("w2", [HID, HID], f32, kind="ExternalInput").ap()
    w3_d = nc.dram_tensor("w3", [HID, HID], f32, kind="ExternalInput").ap()
    w4r_d = nc.dram_tensor("w4r", [HID, 736], f32, kind="ExternalInput").ap()
    b1_d = nc.dram_tensor("b1", [HID, 1], f32, kind="ExternalInput").ap()
    b2_d = nc.dram_tensor("b2", [HID, 1], f32, kind="ExternalInput").ap()
    b3_d = nc.dram_tensor("b3", [HID, 1], f32, kind="ExternalInput").ap()
    b4r_d = nc.dram_tensor("b4r", [1, 736], f32, kind="ExternalInput").ap()
    y2_d = nc.dram_tensor("y2", [rows, DH], f32, kind="ExternalOutput").ap()
    ld_d = nc.dram_tensor("ld", [rows], f32, kind="ExternalOutput").ap()

    x2_v = x2_d.rearrange("(c t p) j -> c p t j", p=128, t=T)
    y2_v = y2_d.rearrange("(c t p) j -> c p t j", p=128, t=T)
    ld_v = ld_d.rearrange("(c t p) -> c p t", p=128, t=T)

    with TileContext(nc) as tc:
        with (
            tc.tile_pool(name="const", bufs=1) as constp,
            tc.tile_pool(name="io", bufs=2) as iop,
            tc.tile_pool(name="mid", bufs=1) as midp,
            tc.tile_pool(name="tail", bufs=1) as tailp,
            tc.tile_pool(name="mlps", bufs=2, space="PSUM") as mlpsp,
            tc.tile_pool(name="spls", bufs=1, space="PSUM") as splsp,
        ):
            # ---- persistent constants ----
            w1s = constp.tile([DH, HID], f32, tag="w1")
            w2s = constp.tile([HID, HID], f32, tag="w2")
            w3s = constp.tile([HID, HID], f32, tag="w3")
            w4s = constp.tile([HID, 736], f32, tag="w4")
            b1s = constp.tile([HID, 1], f32, tag="b1")
            b2s = constp.tile([HID, 1], f32, tag="b2")
            b3s = constp.tile([HID, 1], f32, tag="b3")
            b4s = constp.tile([1, 736], f32, tag="b4")
            ones1 = constp.tile([1, HID], f32, tag="ones1")
            mask01 = constp.tile([128, T * 256], f32, tag="mask01")
            conste = constp.tile([128, T * 256], f32, tag="conste")

            nc.sync.dma_start(w1s[:], w1_d)
            nc.sync.dma_start(w2s[:], w2_d)
            nc.sync.dma_start(w3s[:], w3_d)
            nc.sync.dma_start(w4s[:], w4r_d)
            nc.sync.dma_start(b1s[:], b1_d)
            nc.sync.dma_start(b2s[:], b2_d)
            nc.sync.dma_start(b3s[:], b3_d)
            nc.sync.dma_start(b4s[:], b4r_d)
            nc.gpsimd.memset(ones1[:], 1.0)
            nc.gpsimd.memset(mask01[:], 1.0)
            m4 = mask01[:].rearrange("p (t j k) -> p t j k", t=T, k=K)
            nc.gpsimd.memset(m4[:, :, :, 0:1], 0.0)
            ce4 = conste[:].rearrange("p (t j k) -> p t j k", t=T, k=K)
            for m in range(K):
                # E'_{m+1} const part: 1 + 6*MIN_W*(m+1)  (offset +4 baked in)
                nc.gpsimd.memset(ce4[:, :, :, m:m + 1],
                                 1.0 + 2 * B_TAIL * MIN_W * (m + 1))

            for c in range(n_chunks):
                # ---------------- MLP (feature-major) ----------------
                x1t = iop.tile([DH, CHUNK], f32, tag="x1t")
                nc.sync.dma_start(x1t[:], x1t_d[:, c * CHUNK:(c + 1) * CHUNK])
                x2t = iop.tile([128, T * DH], f32, tag="x2t")
                nc.sync.dma_start(x2t[:].rearrange("p (t j) -> p t j", t=T), x2_v[c])

                hprev = x1t
                wts = [(w1s, b1s), (w2s, b2s), (w3s, b3s)]
                for li, (wv, bv) in enumerate(wts):
                    ps = mlpsp.tile([HID, CHUNK], f32, tag="mlp_ps")
                    nc.tensor.matmul(ps[:], wv[:], hprev[:], start=True, stop=True)
                    epre = iop.tile([HID, CHUNK], f32, tag=f"epre{li}")
                    rel = iop.tile([HID, CHUNK], f32, tag=f"rel{li}")
                    tne = iop.tile([HID, CHUNK], f32, tag=f"tne{li}")
                    h = iop.tile([HID, CHUNK], f32, tag=f"h{li}")
                    nc.scalar.activation(epre[:], ps[:], Act.Exp, bias=bv[:])
                    nc.scalar.activation(rel[:], ps[:], Act.Relu, bias=bv[:])
                    # tne = relu(1 - exp(pre))
                    nc.scalar.activation(tne[:], epre[:], Act.Relu,
                                         bias=1.0, scale=-1.0)
                    nc.vector.tensor_tensor(h[:], rel[:], tne[:], Alu.subtract)
                    hprev = h
                h3 = hprev

                # ---------------- layer 4 -> row-major psum + exp/softplus ----
                ew = midp.tile([128, T * 256], f32, tag="ew")
                eh = midp.tile([128, T * 256], f32, tag="eh")
                d9 = midp.tile([128, T * 288], f32, tag="d9")
                d9v = d9[:].rearrange("p (t j m) -> p t j m", t=T, m=9)
                nc.gpsimd.memset(d9v[:, :, :, 0:1], D_BOUND)
                nc.gpsimd.memset(d9v[:, :, :, 8:9], D_BOUND)

                for t in range(T):
                    psw = splsp.tile([128, 256], f32, tag="psw")
                    psh = splsp.tile([128, 256], f32, tag="psh")
                    psd = splsp.tile([128, 224], f32, tag="psd")
                    h3sl = h3[:, t * 128:(t + 1) * 128]
                    nc.tensor.matmul(psw[:], ones1[:], b4s[:, 0:256],
                                     start=True, stop=False)
                    nc.tensor.matmul(psw[:], h3sl, w4s[:, 0:256],
                                     start=False, stop=True)
                    nc.tensor.matmul(psh[:], ones1[:], b4s[:, 256:512],
                                     start=True, stop=False)
                    nc.tensor.matmul(psh[:], h3sl, w4s[:, 256:512],
                                     start=False, stop=True)
                    nc.tensor.matmul(psd[:], ones1[:], b4s[:, 512:736],
                                     start=True, stop=False)
                    nc.tensor.matmul(psd[:], h3sl, w4s[:, 512:736],
                                     start=False, stop=True)

                    nc.scalar.activation(ew[:, t * 256:(t + 1) * 256], psw[:],
                                         Act.Exp)
                    nc.scalar.activation(eh[:, t * 256:(t + 1) * 256], psh[:],
                                         Act.Exp)
                    spe = iop.tile([128, 224], f32, tag="spe")
                    nc.scalar.activation(spe[:], psd[:], Act.Exp)
                    # softplus = ln(1 + exp)
                    nc.scalar.activation(
                        d9v[:, t, :, 1:8],
                        spe[:].rearrange("p (j m) -> p j m", m=7),
                        Act.Ln, bias=1.0)

                # ---------------- spline (row-major, T row-tiles batched) ----
                NT = T * 256

                def v2(ap):
                    return ap.rearrange("p (t j) -> p t j", t=T)

                def v4(ap):
                    return ap.rearrange("p (t j k) -> p t j k", t=T, k=K)

                x24 = tailp.tile([128, T * DH], f32, tag="x24")
                xc4 = tailp.tile([128, T * DH], f32, tag="xc4")
                nc.vector.tensor_scalar(x24[:], x2t[:], 4.0, None, Alu.add)
                nc.vector.tensor_scalar(xc4[:], x24[:], 7.0, 1.0, Alu.min, Alu.max)

                cumw = midp.tile([128, NT], f32, tag="cumw")
                cumh = midp.tile([128, NT], f32, tag="cumh")
                nc.vector.tensor_tensor_scan(cumw[:], mask01[:], ew[:], 0.0,
                                             Alu.mult, Alu.add)
                nc.vector.tensor_tensor_scan(cumh[:], mask01[:], eh[:], 0.0,
                                             Alu.mult, Alu.add)

                rw = tailp.tile([128, T * DH], f32, tag="rw")
                rh = tailp.tile([128, T * DH], f32, tag="rh")
                nc.vector.reciprocal(v2(rw[:]), v4(cumw[:])[:, :, :, 7])
                nc.vector.reciprocal(v2(rh[:]), v4(cumh[:])[:, :, :, 7])
                rcw = tailp.tile([128, T * DH], f32, tag="rcw")
                rch = tailp.tile([128, T * DH], f32, tag="rch")
                nc.vector.tensor_scalar(rcw[:], rw[:], C_W, None, Alu.mult)
                nc.vector.tensor_scalar(rch[:], rh[:], C_H, None, Alu.mult)

                e9 = midp.tile([128, T * 288], f32, tag="e9")
                ch9 = midp.tile([128, T * 288], f32, tag="ch9")
                e9v = e9[:].rearrange("p (t j m) -> p t j m", t=T, m=9)
                ch9v = ch9[:].rearrange("p (t j m) -> p t j m", t=T, m=9)
                nc.gpsimd.memset(e9v[:, :, :, 0:1], 1.0)
                nc.gpsimd.memset(ch9v[:, :, :, 0:1], 1.0)

                tw = midp.tile([128, NT], f32, tag="tw")
                th = midp.tile([128, NT], f32, tag="th")

                def bc8(ap):  # [128, T*32] -> broadcast over k
                    return v2(ap).unsqueeze(3).broadcast_to([128, T, DH, K])

                nc.vector.tensor_tensor(v4(tw[:]), v4(cumw[:]), bc8(rcw[:]),
                                        Alu.mult)
                nc.vector.tensor_tensor(e9v[:, :, :, 1:9], v4(tw[:]), ce4,
                                        Alu.add)
                nc.vector.tensor_tensor(v4(th[:]), v4(cumh[:]), bc8(rch[:]),
                                        Alu.mult)
                nc.vector.tensor_tensor(ch9v[:, :, :, 1:9], v4(th[:]), ce4,
                                        Alu.add)

                lt = midp.tile([128, NT], f32, tag="lt")
                nc.vector.tensor_tensor(v4(lt[:]), bc8(xc4[:]),
                                        e9v[:, :, :, 0:8], Alu.is_lt)

                # six select-scan gathers
                gouts = []
                for name, src in (("gel", e9v[:, :, :, 0:8]),
                                  ("geh", e9v[:, :, :, 1:9]),
                                  ("gcl", ch9v[:, :, :, 0:8]),
                                  ("gch", ch9v[:, :, :, 1:9]),
                                  ("gdl", d9v[:, :, :, 0:8]),
                                  ("gdh", d9v[:, :, :, 1:9])):
                    d1 = midp.tile([128, NT], f32, tag=f"d1{name}")
                    g = midp.tile([128, NT], f32, tag=f"g{name}")
                    nc.vector.scalar_tensor_tensor(v4(d1[:]), v4(lt[:]), -BIG,
                                                   src, Alu.mult, Alu.add)
                    nc.vector.tensor_tensor_scan(g[:], lt[:], d1[:], 0.0,
                                                 Alu.mult, Alu.max)
                    gouts.append(v4(g[:])[:, :, :, 7])
                gel, geh, gcl, gch, gdl, gdh = gouts

                # ---------------- tail ----------------
                def tl(tag):
                    tile = tailp.tile([128, T * DH], f32, tag=tag)
                    return tile, v2(tile[:])

                u_t, u = tl("u")
                v_t, v = tl("v")
                w_t, w = tl("w")
                h_t, h = tl("h")
                xc4v = v2(xc4[:])
                nc.vector.tensor_tensor(u, xc4v, gel, Alu.subtract)
                nc.vector.tensor_tensor(v, geh, xc4v, Alu.subtract)
                nc.vector.tensor_tensor(w, geh, gel, Alu.subtract)
                nc.vector.tensor_tensor(h, gch, gcl, Alu.subtract)

                u2_t, u2 = tl("u2")
                vsq_t, vsq = tl("vsq")
                w2_t, w2 = tl("w2")
                h2_t, h2 = tl("h2")
                nc.scalar.activation(u2, u, Act.Square)
                nc.scalar.activation(vsq, v, Act.Square)
                nc.scalar.activation(w2, w, Act.Square)
                nc.scalar.activation(h2, h, Act.Square)

                p_t, p = tl("p")
                pw_t, pw = tl("pw")
                nc.vector.tensor_tensor(p, u, v, Alu.mult)
                nc.vector.tensor_tensor(pw, p, w, Alu.mult)

                hu2_t, hu2 = tl("hu2")
                dpw_t, dpw = tl("dpw")
                nc.vector.tensor_tensor(hu2, h, u2, Alu.mult)
                nc.vector.scalar_tensor_tensor(dpw, gdl, MIN_D, pw,
                                               Alu.add, Alu.mult)
                s_t, s = tl("s")
                num_t, num = tl("num")
                nc.vector.tensor_tensor(s, hu2, dpw, Alu.add)
                nc.vector.tensor_tensor(num, h, s, Alu.mult)

                hp_t, hp = tl("hp")
                hw2_t, hw2 = tl("hw2")
                nc.vector.tensor_tensor(hp, h, p, Alu.mult)
                nc.vector.tensor_tensor(hw2, h, w2, Alu.mult)
                ds_t, ds = tl("ds")
                t3_t, t3 = tl("t3")
                nc.vector.tensor_tensor(ds, gdl, gdh, Alu.add)
                nc.vector.scalar_tensor_tensor(t3, ds, 2 * MIN_D, pw,
                                               Alu.add, Alu.mult)
                t4_t, t4 = tl("t4")
                den_t, den = tl("den")
                nc.vector.scalar_tensor_tensor(t4, hp, -2.0, hw2,
                                               Alu.mult, Alu.add)
                nc.vector.tensor_tensor(den, t4, t3, Alu.add)

                rden_t, rden = tl("rden")
                nc.vector.reciprocal(rden, den)
                ratio_t, ratio = tl("ratio")
                nc.vector.tensor_tensor(ratio, num, rden, Alu.mult)
                yin_t, yin = tl("yin")
                nc.vector.scalar_tensor_tensor(yin, ratio, -4.0, gcl,
                                               Alu.add, Alu.add)

                # log-det: ld = ln(h^2 * G * w) - 2 ln(den)
                # G = d1*u2*w + d*vsq*w + 2*h*p
                t5_t, t5 = tl("t5")
                t6_t, t6 = tl("t6")
                nc.vector.scalar_tensor_tensor(t5, gdh, MIN_D, u2,
                                               Alu.add, Alu.mult)
                nc.vector.scalar_tensor_tensor(t6, gdl, MIN_D, vsq,
                                               Alu.add, Alu.mult)
                g0_t, g0 = tl("g0")
                gw_t, gw = tl("gw")
                gg_t, gg = tl("gg")
                nc.vector.tensor_tensor(g0, t5, t6, Alu.add)
                nc.vector.tensor_tensor(gw, g0, w, Alu.mult)
                nc.vector.scalar_tensor_tensor(gg, hp, 2.0, gw,
                                               Alu.mult, Alu.add)
                t9_t, t9 = tl("t9")
                harg_t, harg = tl("harg")
                nc.vector.tensor_tensor(t9, h2, gg, Alu.mult)
                nc.vector.tensor_tensor(harg, t9, w, Alu.mult)
                l1_t, l1 = tl("l1")
                l2_t, l2 = tl("l2")
                nc.scalar.activation(l1, harg, Act.Ln)
                nc.scalar.activation(l2, den, Act.Ln)
                ldin_t, ldin = tl("ldin")
                nc.vector.scalar_tensor_tensor(ldin, l2, -2.0, l1,
                                               Alu.mult, Alu.add)

                # inside mask + select
                mu8 = tailp.tile([128, T * DH], u8, tag="mu8")
                nc.vector.tensor_tensor(mu8[:], x24[:], xc4[:], Alu.is_equal)
                yout = tailp.tile([128, T * DH], f32, tag="yout")
                nc.vector.tensor_copy(yout[:], x2t[:])
                nc.vector.copy_predicated(v2(yout[:]), v2(mu8[:]), yin)
                ldz = tailp.tile([128, T * DH], f32, tag="ldz")
                nc.gpsimd.memset(ldz[:], 0.0)
                nc.vector.copy_predicated(v2(ldz[:]), v2(mu8[:]), ldin)
                ldt = tailp.tile([128, T], f32, tag="ldt")
                nc.vector.tensor_reduce(ldt[:], v2(ldz[:]),
                                        mybir.AxisListType.X, Alu.add)

                nc.sync.dma_start(y2_v[c], yout[:].rearrange("p (t j) -> p t j", t=T))
                nc.sync.dma_start(ld_v[c], ldt[:])

    nc.compile()
    return nc


def _prep_weights(W1, b1, W2, b2, W3, b3, W4, b4):
    # W4 columns: orig layout j*23 + {0..7 -> uw, 8..15 -> uh, 16..22 -> ud}
    W4 = np.asarray(W4, np.float32).reshape(HID, DH, 23)
    b4 = np.asarray(b4, np.float32).reshape(DH, 23)
    w4r = np.concatenate(
        [W4[:, :, 0:8].reshape(HID, -1),
         W4[:, :, 8:16].reshape(HID, -1),
         W4[:, :, 16:23].reshape(HID, -1)], axis=1)
    b4r = np.concatenate(
        [b4[:, 0:8].reshape(-1), b4[:, 8:16].reshape(-1),
         b4[:, 16:23].reshape(-1)])[None, :]
    return {
        "w1": np.ascontiguousarray(W1, np.float32),
        "w2": np.ascontiguousarray(W2, np.float32),
        "w3": np.ascontiguousarray(W3, np.float32),
        "w4r": np.ascontiguousarray(w4r),
        "b1": np.asarray(b1, np.float32).reshape(HID, 1),
        "b2": np.asarray(b2, np.float32).reshape(HID, 1),
        "b3": np.asarray(b3, np.float32).reshape(HID, 1),
        "b4r": np.ascontiguousarray(b4r),
    }


def kernel(x, W1, b1, W2, b2, W3, b3, W4, b4, _collect=None):
    from concourse.bass_utils import run_bass_kernel_spmd

    x = np.asarray(x, np.float32)
    wmap = _prep_weights(W1, b1, W2, b2, W3, b3, W4, b4)

    if "nc" not in _CACHE:
        _CACHE["nc"] = _build()
    nc = _CACHE["nc"]

    in_maps = []
    for cid in range(N_CORES):
        xs = x[cid * R:(cid + 1) * R]
        m = dict(wmap)
        m["x1t"] = np.ascontiguousarray(xs[:, :DH].T)
        m["x2"] = np.ascontiguousarray(xs[:, DH:])
        in_maps.append(m)

    res = run_bass_kernel_spmd(nc, in_maps, list(range(N_CORES)))
    if _collect is not None:
        _collect.append(res)

    y2 = np.concatenate([r["y2"] for r in res.results], axis=0)
    ld = np.concatenate([r["ld"] for r in res.results], axis=0)
    out = np.concatenate([x[:, :DH], y2], axis=1)
    return out, ld


# revision 9
# speedup vs baseline: 1.2955x; 1.2955x over previous
"""Trainium2 Bass kernel for nn_CouplingNSF (coupling-layer neural spline flow).

Contract: kernel(**inputs) takes the FULL inputs (x [131072,64], W1..W4, b1..b4)
and returns (concat([x1, y2], axis=1) [131072,64] float32, log_det [131072]
float32), matching the reference module.

Strategy: pure data-parallel over 8 NeuronCores (16384 rows each).
 - MLP runs feature-major (activations [features, rows]) so weights are the
   stationary matmul operand and no transposes are needed between layers;
   x1^T is prepared on the host.  The last layer uses lhsT = h3^T which flips
   the output to row-major [rows, 736]; b4 is added with a contraction-1
   ones-matmul accumulated into the same PSUM tile.  All matmuls run as
   float32r (full-rate PE streaming; fp32 proper is 4 cycles/row).
 - ELU is computed as elu(x) = min(exp(x),1) + relu(x) - 1 with the constant
   -1 folded into the next layer's bias (b' = b - colsum(W)), so each hidden
   layer is two accumulating matmuls and the only vector op is a 2x-mode min.
 - Rational-quadratic spline per (row, dim) in row-major layout:
   cumsum over the 8 bins via tensor_tensor_scan with a 0/1 reset mask, bin
   search via one broadcast compare, one-hot = adjacent difference of the
   compare, and all gathers are (one-hot * raw-cumsum) + per-group
   tensor_reduce; edges are reconstructed from the gathered raw cumsums, the
   bin count and the softmax normalizer, so no per-bin edge tensors are ever
   materialized.
 - All scalar-engine functions (Exp, Ln, Relu, Square, Copy/Identity) live in
   the one ACT table set "natural_log_exp_and_others"; get_activation_tables
   is patched so the compiler can only pick that set (otherwise it thrashes
   ACT_TABLE_LOADs on every Exp<->Ln transition).
"""

import numpy as np

DIM = 64
DH = 32          # d_half
K = 8
B_TAIL = 3.0
HID = 128
BATCH = 131072
MIN_W = 1e-3
MIN_H = 1e-3
MIN_D = 1e-3
N_CORES = 8
R = BATCH // N_CORES          # rows per core = 16384
T = 4                         # row-tiles (of 128) per chunk
CHUNK = 128 * T               # 512 rows
N_CHUNKS = R // CHUNK         # 32

C_W = 2 * B_TAIL * (1.0 - MIN_W * K)   # 6*(1-8e-3)
C_H = 2 * B_TAIL * (1.0 - MIN_H * K)
SM = 2 * B_TAIL * MIN_W                # 6*MIN per-bin constant edge step
D_BOUND = 1.0 - MIN_D                  # stored boundary value (MIN added post-gather)
W4C = 768                              # ud block padded 224 -> 256

_CACHE = {}
_PATCHED = []


def _patch_act_tables():
    """Restrict every ACT function we use to natural_log_exp_and_others so
    exactly one ACT_TABLE_LOAD is emitted instead of per-transition thrash."""
    if _PATCHED:
        return
    import concourse.bacc as bacc_mod

    orig = bacc_mod.get_activation_tables

    def patched(arch):
        tabs = orig(arch)
        keep = "natural_log_exp_and_others"
        mine = tabs[keep]
        return {n: (fns if n == keep else (fns - mine))
                for n, fns in tabs.items()}

    bacc_mod.get_activation_tables = patched
    _PATCHED.append(True)


def _build(n_chunks=N_CHUNKS):
    import concourse.bass as bass
    from concourse import bacc, mybir
    from concourse.tile import TileContext

    _patch_act_tables()

    f32 = mybir.dt.float32
    f32r = mybir.dt.float32r
    u8 = mybir.dt.uint8
    Alu = mybir.AluOpType
    Act = mybir.ActivationFunctionType
    AxX = mybir.AxisListType.X

    rows = n_chunks * CHUNK

    nc = bacc.Bacc("TRN2", target_bir_lowering=False, debug=False,
                   num_devices=N_CORES)

    x1t_d = nc.dram_tensor("x1t", [DH, rows], f32, kind="ExternalInput").ap()
    x2_d = nc.dram_tensor("x2", [rows, DH], f32, kind="ExternalInput").ap()
    w1_d = nc.dram_tensor("w1", [DH, HID], f32, kind="ExternalInput").ap()
    w2_d = nc.dram_tensor("w2", [HID, HID], f32, kind="ExternalInput").ap()
    w3_d = nc.dram_tensor("w3", [HID, HID], f32, kind="ExternalInput").ap()
    w4r_d = nc.dram_tensor("w4r", [HID, W4C], f32, kind="ExternalInput").ap()
    b1_d = nc.dram_tensor("b1", [HID, 1], f32, kind="ExternalInput").ap()
    b2_d = nc.dram_tensor("b2", [HID, 1], f32, kind="ExternalInput").ap()
    b3_d = nc.dram_tensor("b3", [HID, 1], f32, kind="ExternalInput").ap()
    b4r_d = nc.dram_tensor("b4r", [1, W4C], f32, kind="ExternalInput").ap()
    ones_d = nc.dram_tensor("onesv", [1, HID], f32, kind="ExternalInput").ap()
    y2_d = nc.dram_tensor("y2", [rows, DH], f32, kind="ExternalOutput").ap()
    ld_d = nc.dram_tensor("ld", [rows], f32, kind="ExternalOutput").ap()

    x2_v = x2_d.rearrange("(c t p) j -> c p t j", p=128, t=T)
    y2_v = y2_d.rearrange("(c t p) j -> c p t j", p=128, t=T)
    ld_v = ld_d.rearrange("(c t p) -> c p t", p=128, t=T)

    NT = T * 256

    with TileContext(nc) as tc:
        with (
            tc.tile_pool(name="const", bufs=1) as constp,
            tc.tile_pool(name="io", bufs=2) as iop,
            tc.tile_pool(name="mid", bufs=2) as midp,
            tc.tile_pool(name="tail", bufs=1) as tailp,
            tc.tile_pool(name="mlps", bufs=2, space="PSUM") as mlpsp,
            tc.tile_pool(name="spls", bufs=2, space="PSUM") as splsp,
        ):
            # ---- persistent constants ----
            w1s = constp.tile([DH, HID], f32, tag="w1")
            w2s = constp.tile([HID, HID], f32, tag="w2")
            w3s = constp.tile([HID, HID], f32, tag="w3")
            w4s = constp.tile([HID, W4C], f32, tag="w4")
            b1s = constp.tile([HID, 1], f32, tag="b1")
            b2s = constp.tile([HID, 1], f32, tag="b2")
            b3s = constp.tile([HID, 1], f32, tag="b3")
            b4s = constp.tile([1, W4C], f32, tag="b4")
            ones1 = constp.tile([1, HID], f32, tag="ones1")
            mask01 = constp.tile([128, NT], f32, tag="mask01")
            idx9 = constp.tile([128, T * 288], f32, tag="idx9")

            nc.sync.dma_start(w1s[:], w1_d)
            nc.sync.dma_start(w2s[:], w2_d)
            nc.sync.dma_start(w3s[:], w3_d)
            nc.sync.dma_start(w4s[:], w4r_d)
            nc.sync.dma_start(b1s[:], b1_d)
            nc.sync.dma_start(b2s[:], b2_d)
            nc.sync.dma_start(b3s[:], b3_d)
            nc.sync.dma_start(b4s[:], b4r_d)
            nc.sync.dma_start(ones1[:], ones_d)
            nc.gpsimd.memset(mask01[:], 1.0)
            m4 = mask01[:].rearrange("p (t j k) -> p t j k", t=T, k=K)
            nc.gpsimd.memset(m4[:, :, :, 0:1], 0.0)
            ix4 = idx9[:].rearrange("p (t j m) -> p t j m", t=T, m=9)
            for m in range(9):
                nc.gpsimd.memset(ix4[:, :, :, m:m + 1], SM * m)

            def v2(ap):
                return ap.rearrange("p (t j) -> p t j", t=T)

            def v4(ap):
                return ap.rearrange("p (t j k) -> p t j k", t=T, k=K)

            for c in range(n_chunks):
                # ---------------- MLP (feature-major) ----------------
                x1t = iop.tile([DH, CHUNK], f32, tag="x1t")
                nc.sync.dma_start(x1t[:], x1t_d[:, c * CHUNK:(c + 1) * CHUNK])
                x2t = iop.tile([128, T * DH], f32, tag="x2t")
                nc.sync.dma_start(x2t[:].rearrange("p (t j) -> p t j", t=T),
                                  x2_v[c])

                # layer 1 (plain input)
                ps = mlpsp.tile([HID, CHUNK], f32, tag="mlp_ps")
                nc.tensor.matmul(ps[:], w1s[:], x1t[:],
                                 start=True, stop=True)
                # elu(pre) = min(exp(pre),1) + relu(pre) - 1, the -1 folded
                # into the next bias on the host; sum done on DVE.
                def elu(ps, bv, tag):
                    ea = iop.tile([HID, CHUNK], f32, tag=f"ea{tag}")
                    h = iop.tile([HID, CHUNK], f32, tag=f"h{tag}")
                    nc.scalar.activation(ea[:], ps[:], Act.Exp, bias=bv[:])
                    nc.vector.tensor_scalar(ea[:], ea[:], 1.0, None, Alu.min)
                    nc.scalar.activation(h[:], ps[:], Act.Relu, bias=bv[:])
                    nc.vector.tensor_tensor(h[:], h[:], ea[:], Alu.add)
                    return h
                h1 = elu(ps, b1s, "1")
                ps = mlpsp.tile([HID, CHUNK], f32, tag="mlp_ps")
                nc.tensor.matmul(ps[:], w2s[:], h1[:], start=True, stop=True)
                h2 = elu(ps, b2s, "2")
                ps = mlpsp.tile([HID, CHUNK], f32, tag="mlp_ps")
                nc.tensor.matmul(ps[:], w3s[:], h2[:], start=True, stop=True)
                h3 = elu(ps, b3s, "3")

                # ---------------- layer 4 -> row-major psum + exp/softplus --
                ew = midp.tile([128, NT], f32, tag="ew")
                eh = midp.tile([128, NT], f32, tag="eh")
                d9 = midp.tile([128, T * 288], f32, tag="d9")
                d9v = d9[:].rearrange("p (t j m) -> p t j m", t=T, m=9)
                nc.gpsimd.memset(d9v[:, :, :, 0:1], D_BOUND)
                nc.gpsimd.memset(d9v[:, :, :, 8:9], D_BOUND)

                for t in range(T):
                    pswh = splsp.tile([128, 512], f32, tag="pswh")
                    psd = splsp.tile([128, 256], f32, tag="psd")
                    h3sl = h3[:, t * 128:(t + 1) * 128]
                    nc.tensor.matmul(pswh[:], ones1[:], b4s[:, 0:512],
                                     start=True, stop=False)
                    nc.tensor.matmul(pswh[:], h3sl, w4s[:, 0:512],
                                     start=False, stop=True)
                    nc.tensor.matmul(psd[:], ones1[:], b4s[:, 512:768],
                                     start=True, stop=False)
                    nc.tensor.matmul(psd[:], h3sl, w4s[:, 512:768],
                                     start=False, stop=True)

                    nc.scalar.activation(ew[:, t * 256:(t + 1) * 256],
                                         pswh[:, 0:256], Act.Exp)
                    nc.scalar.activation(eh[:, t * 256:(t + 1) * 256],
                                         pswh[:, 256:512], Act.Exp)
                    spe = iop.tile([128, 224], f32, tag="spe")
                    nc.scalar.activation(spe[:], psd[:, 0:224], Act.Exp)
                    # softplus = ln(1 + exp)
                    nc.scalar.activation(
                        d9v[:, t, :, 1:8],
                        spe[:].rearrange("p (j m) -> p j m", m=7),
                        Act.Ln, bias=1.0)

                # ---------------- spline ----------------
                xc = tailp.tile([128, T * DH], f32, tag="xc")
                nc.vector.tensor_scalar(xc[:], x2t[:], B_TAIL, -B_TAIL,
                                        Alu.min, Alu.max)
                xc3 = tailp.tile([128, T * DH], f32, tag="xc3")
                nc.vector.tensor_scalar(xc3[:], xc[:], B_TAIL, None, Alu.add)

                cumw = midp.tile([128, NT], f32, tag="cumw")
                cumh = midp.tile([128, NT], f32, tag="cumh")
                nc.vector.tensor_tensor_scan(cumw[:], mask01[:], ew[:], 0.0,
                                             Alu.mult, Alu.add)
                nc.vector.tensor_tensor_scan(cumh[:], mask01[:], eh[:], 0.0,
                                             Alu.mult, Alu.add)

                rw = tailp.tile([128, T * DH], f32, tag="rw")
                rh = tailp.tile([128, T * DH], f32, tag="rh")
                nc.vector.reciprocal(v2(rw[:]), v4(cumw[:])[:, :, :, 7])
                nc.vector.reciprocal(v2(rh[:]), v4(cumh[:])[:, :, :, 7])
                rcw = tailp.tile([128, T * DH], f32, tag="rcw")
                rch = tailp.tile([128, T * DH], f32, tag="rch")
                nc.vector.tensor_scalar(rcw[:], rw[:], C_W, None, Alu.mult)
                nc.vector.tensor_scalar(rch[:], rh[:], C_H, None, Alu.mult)

                # shifted-edge arrays (origin-shifted by +3):
                # et9_m = SM*m + rcw*S_{m-1} = e_m + 3, m = 0..8
                et9 = midp.tile([128, T * 288], f32, tag="et9")
                ct9 = midp.tile([128, T * 288], f32, tag="ct9")
                e9v = et9[:].rearrange("p (t j m) -> p t j m", t=T, m=9)
                c9v = ct9[:].rearrange("p (t j m) -> p t j m", t=T, m=9)
                i9v = idx9[:].rearrange("p (t j m) -> p t j m", t=T, m=9)
                nc.gpsimd.memset(e9v[:, :, :, 0:1], 0.0)
                nc.gpsimd.memset(c9v[:, :, :, 0:1], 0.0)
                rcwb = v2(rcw[:]).unsqueeze(3).broadcast_to([128, T, DH, K])
                rchb = v2(rch[:]).unsqueeze(3).broadcast_to([128, T, DH, K])
                nc.vector.tensor_tensor(e9v[:, :, :, 1:9], v4(cumw[:]), rcwb,
                                        Alu.mult)
                nc.vector.tensor_tensor(e9v[:, :, :, 1:9], e9v[:, :, :, 1:9],
                                        i9v[:, :, :, 1:9], Alu.add)
                nc.vector.tensor_tensor(c9v[:, :, :, 1:9], v4(cumh[:]), rchb,
                                        Alu.mult)
                nc.vector.tensor_tensor(c9v[:, :, :, 1:9], c9v[:, :, :, 1:9],
                                        i9v[:, :, :, 1:9], Alu.add)

                # lt_m = xc < e_m  <=>  xc3 < et9_m (m = 0..7)
                lt = midp.tile([128, NT], f32, tag="lt")
                xc3b = v2(xc3[:]).unsqueeze(3).broadcast_to([128, T, DH, K])
                nc.vector.tensor_tensor(v4(lt[:]), xc3b, e9v[:, :, :, 0:8],
                                        Alu.is_lt)
                # one-hot of the bin: oh_k = lt_{k+1} - lt_k (lt_8 == 1)
                oh = midp.tile([128, NT], f32, tag="oh")
                o4 = v4(oh[:])
                l4 = v4(lt[:])
                nc.vector.tensor_tensor(o4[:, :, :, 0:7], l4[:, :, :, 1:8],
                                        l4[:, :, :, 0:7], Alu.subtract)
                nc.vector.tensor_scalar(o4[:, :, :, 7], l4[:, :, :, 7],
                                        -1.0, 1.0, Alu.mult, Alu.add)

                def gather(name, src):
                    prod = midp.tile([128, NT], f32, tag="prod")
                    g = tailp.tile([128, T * DH], f32, tag=f"g{name}")
                    p4 = v4(prod[:])
                    nc.vector.tensor_tensor(p4, o4, src, Alu.mult)
                    nc.vector.tensor_reduce(v2(g[:]), p4, AxX, Alu.add)
                    return v2(g[:])

                glo = gather("lo", e9v[:, :, :, 0:8])   # e_b + 3
                ghi = gather("hi", e9v[:, :, :, 1:9])   # e_{b+1} + 3
                gcl = gather("cl", c9v[:, :, :, 0:8])   # ch_b + 3
                gch = gather("ch", c9v[:, :, :, 1:9])   # ch_{b+1} + 3
                gdl = gather("dl", d9v[:, :, :, 0:8])   # d_b - MIN
                gdh = gather("dh", d9v[:, :, :, 1:9])   # d_{b+1} - MIN

                # ---------------- tail ----------------
                def tl(tag):
                    tile = tailp.tile([128, T * DH], f32, tag=tag)
                    return tile, v2(tile[:])

                xc3v = v2(xc3[:])
                u_t, u = tl("u")
                vv_t, vv = tl("vv")
                w_t, w = tl("w")
                h_t, h = tl("h")
                nc.vector.tensor_tensor(u, xc3v, glo, Alu.subtract)
                nc.vector.tensor_tensor(vv, ghi, xc3v, Alu.subtract)
                nc.vector.tensor_tensor(w, ghi, glo, Alu.subtract)
                nc.vector.tensor_tensor(h, gch, gcl, Alu.subtract)

                u2_t, u2 = tl("u2")
                vsq_t, vsq = tl("vsq")
                w2_t, w2 = tl("w2")
                h2_t, h2 = tl("h2")
                nc.scalar.activation(u2, u, Act.Square)
                nc.scalar.activation(vsq, vv, Act.Square)
                nc.scalar.activation(w2, w, Act.Square)
                nc.scalar.activation(h2, h, Act.Square)

                p_t, p = tl("p")
                pw_t, pw = tl("pw")
                nc.vector.tensor_tensor(p, u, vv, Alu.mult)
                nc.vector.tensor_tensor(pw, p, w, Alu.mult)

                hu2_t, hu2 = tl("hu2")
                dpw_t, dpw = tl("dpw")
                nc.vector.tensor_tensor(hu2, h, u2, Alu.mult)
                nc.vector.scalar_tensor_tensor(dpw, gdl, MIN_D, pw,
                                               Alu.add, Alu.mult)
                s_t, s = tl("s")
                num_t, num = tl("num")
                nc.vector.tensor_tensor(s, hu2, dpw, Alu.add)
                nc.vector.tensor_tensor(num, h, s, Alu.mult)

                hp_t, hp = tl("hp")
                hw2_t, hw2 = tl("hw2")
                nc.vector.tensor_tensor(hp, h, p, Alu.mult)
                nc.vector.tensor_tensor(hw2, h, w2, Alu.mult)
                ds_t, ds = tl("ds")
                t3_t, t3 = tl("t3")
                nc.vector.tensor_tensor(ds, gdl, gdh, Alu.add)
                nc.vector.scalar_tensor_tensor(t3, ds, 2 * MIN_D, pw,
                                               Alu.add, Alu.mult)
                t4_t, t4 = tl("t4")
                den_t, den = tl("den")
                nc.vector.scalar_tensor_tensor(t4, hp, -2.0, hw2,
                                               Alu.mult, Alu.add)
                nc.vector.tensor_tensor(den, t4, t3, Alu.add)

                rden_t, rden = tl("rden")
                nc.vector.reciprocal(rden, den)
                ratio_t, ratio = tl("ratio")
                nc.vector.tensor_tensor(ratio, num, rden, Alu.mult)
                # yin = ratio + ch_b = ratio + (gcl - 3)
                yin_t, yin = tl("yin")
                nc.vector.scalar_tensor_tensor(yin, gcl, -B_TAIL, ratio,
                                               Alu.add, Alu.add)

                # log-det: ld = ln(h^2 * G * w * rden^2)
                # G = d1*u2*w + d*vsq*w + 2*h*p
                t5_t, t5 = tl("t5")
                t6_t, t6 = tl("t6")
                nc.vector.scalar_tensor_tensor(t5, gdh, MIN_D, u2,
                                               Alu.add, Alu.mult)
                nc.vector.scalar_tensor_tensor(t6, gdl, MIN_D, vsq,
                                               Alu.add, Alu.mult)
                g0_t, g0 = tl("g0")
                gw_t, gw = tl("gw")
                gg_t, gg = tl("gg")
                nc.vector.tensor_tensor(g0, t5, t6, Alu.add)
                nc.vector.tensor_tensor(gw, g0, w, Alu.mult)
                nc.vector.scalar_tensor_tensor(gg, hp, 2.0, gw,
                                               Alu.mult, Alu.add)
                rd2_t, rd2 = tl("rd2")
                nc.scalar.activation(rd2, rden, Act.Square)
                t9_t, t9 = tl("t9")
                t10_t, t10 = tl("t10")
                harg_t, harg = tl("harg")
                nc.vector.tensor_tensor(t9, h2, gg, Alu.mult)
                nc.vector.tensor_tensor(t10, t9, w, Alu.mult)
                nc.vector.tensor_tensor(harg, t10, rd2, Alu.mult)
                ldin_t, ldin = tl("ldin")
                nc.scalar.activation(ldin, harg, Act.Ln)

                # inside mask + select
                mu8 = tailp.tile([128, T * DH], u8, tag="mu8")
                nc.vector.tensor_tensor(mu8[:], x2t[:], xc[:], Alu.is_equal)
                yout = tailp.tile([128, T * DH], f32, tag="yout")
                nc.vector.tensor_copy(yout[:], x2t[:])
                nc.vector.copy_predicated(v2(yout[:]), v2(mu8[:]), yin)
                ldz = tailp.tile([128, T * DH], f32, tag="ldz")
                nc.gpsimd.memset(ldz[:], 0.0)
                nc.vector.copy_predicated(v2(ldz[:]), v2(mu8[:]), ldin)
                ldt = tailp.tile([128, T], f32, tag="ldt")
                nc.vector.tensor_reduce(ldt[:], v2(ldz[:]), AxX, Alu.add)

                nc.sync.dma_start(y2_v[c],
                                  yout[:].rearrange("p (t j) -> p t j", t=T))
                nc.sync.dma_start(ld_v[c], ldt[:])

    nc.compile()
    return nc


def _prep_weights(W1, b1, W2, b2, W3, b3, W4, b4):
    # W4 columns: orig layout j*23 + {0..7 -> uw, 8..15 -> uh, 16..22 -> ud}
    W1 = np.asarray(W1, np.float32)
    W2 = np.asarray(W2, np.float32)
    W3 = np.asarray(W3, np.float32)
    W4 = np.asarray(W4, np.float32).reshape(HID, DH, 23)
    b4 = np.asarray(b4, np.float32).reshape(DH, 23)
    w4r = np.zeros((HID, W4C), np.float32)
    w4r[:, 0:256] = W4[:, :, 0:8].reshape(HID, -1)
    w4r[:, 256:512] = W4[:, :, 8:16].reshape(HID, -1)
    w4r[:, 512:736] = W4[:, :, 16:23].reshape(HID, -1)
    b4r = np.zeros((1, W4C), np.float32)
    b4r[0, 0:256] = b4[:, 0:8].reshape(-1)
    b4r[0, 256:512] = b4[:, 8:16].reshape(-1)
    b4r[0, 512:736] = b4[:, 16:23].reshape(-1)
    # elu(x) = min(exp(x),1) + relu(x) - 1 : fold the -1 into the next bias
    b2f = np.asarray(b2, np.float32) - W2.sum(axis=0)
    b3f = np.asarray(b3, np.float32) - W3.sum(axis=0)
    b4r = b4r - w4r.sum(axis=0, keepdims=True)
    return {
        "w1": np.ascontiguousarray(W1),
        "w2": np.ascontiguousarray(W2),
        "w3": np.ascontiguousarray(W3),
        "w4r": np.ascontiguousarray(w4r),
        "b1": np.asarray(b1, np.float32).reshape(HID, 1).copy(),
        "b2": b2f.reshape(HID, 1).copy(),
        "b3": b3f.reshape(HID, 1).copy(),
        "b4r": np.ascontiguousarray(b4r),
        "onesv": np.ones((1, HID), np.float32),
    }


def kernel(x, W1, b1, W2, b2, W3, b3, W4, b4, _collect=None):
    from concourse.bass_utils import run_bass_kernel_spmd

    x = np.asarray(x, np.float32)
    wmap = _prep_weights(W1, b1, W2, b2, W3, b3, W4, b4)

    if "nc" not in _CACHE:
        _CACHE["nc"] = _build()
    nc = _CACHE["nc"]

    in_maps = []
    for cid in range(N_CORES):
        xs = x[cid * R:(cid + 1) * R]
        m = dict(wmap)
        m["x1t"] = np.ascontiguousarray(xs[:, :DH].T)
        m["x2"] = np.ascontiguousarray(xs[:, DH:])
        in_maps.append(m)

    res = run_bass_kernel_spmd(nc, in_maps, list(range(N_CORES)))
    if _collect is not None:
        _collect.append(res)

    y2 = np.concatenate([r["y2"] for r in res.results], axis=0)
    ld = np.concatenate([r["ld"] for r in res.results], axis=0)
    out = np.concatenate([x[:, :DH], y2], axis=1)
    return out, ld
